# revision 19
# baseline (speedup 1.0000x reference)
"""Trainium2 Bass kernel for nn_AttentionValueIteration.

Data-parallel over batch B=8: one batch element per NeuronCore (8 cores).

Per-core pipeline (T=H=W=32, A=8, D=8, K=3, P=1), processed in 8 slabs of
4 t-slices:
  1. x = values + rewards -> x_pad [32 h-partitions, (8 t, 32 w)] fp16
     (t halo +-2; out-of-range t slices stay zero).
  2. im2col via 27 shifted SBUF->SBUF DMAs -> cols [27, 6*1024] fp16
     (borders provided by a memset; DMAs copy the valid subwindow).
  3. qk conv + v conv as PE matmuls over cols (contract dim 27).
     Evictions build:
       q2  [128 = 2 replicas x 64ch, 4096]          (fp16)
       k2  [128, 6*34*34]  rows 0:64 = padded k, rows 64:128 = k shifted
           by +1 flat element (so one TT covers offset pairs (dw=-1, dw=0))
       v_pad [8, 6*34*34]  padded v                  (bf16)
  4. sim: 14 "passes" cover the 27 neighborhood offsets (9 single-TT pairs
     via the k2 replica trick, 4 two-TT pairs, 1 solo).  Each pass:
     DVE TT prod = q2 * shifted(k2)  [*, 512-chunk], then a PE matmul with
     a constant block-ones lhsT reduces over d (partition groups) into a
     per-round PSUM tile [128, 512] (4 passes -> 4 disjoint 32-row blocks).
  5. Softmax without max-subtraction (|sim| stays well within exp range):
     ACT evicts PSUM with fused Exp -> E [128, 512] bf16.  S = sum_o E and
     qv_raw = sum_o E*vn via PE matmuls with block-ones lhsT accumulating
     over the 4 rounds in PSUM.  vn (neighborhoods of v) built by 32
     slab-wide DMAs from v_pad into persistent, once-zeroed tiles.
  6. qv = qv_raw * reciprocal(S); final max over the 8 action partitions via
     gpsimd partition_all_reduce; DMA row 0 to the output.
"""
import sys

sys.path.insert(0, "/opt/trn_rl_repo")

import numpy as np

B, P, A, D, K = 8, 1, 8, 8, 3
T, H, W = 32, 32, 32
PA = P * A
THW = T * H * W
K3 = 27

TS = 4                    # output t-slices per slab
NSLAB = T // TS           # 8
CONVS = TS + 2            # conv slices per slab (halo +-1)
SLABP = TS * H * W        # 4096 positions per slab
NCHUNK = SLABP // 512     # 8
PADHW = 34 * 34
PADVOL = CONVS * PADHW    # 6936
COLSN = CONVS * H * W     # 6144
XPW = 64 + 1024 + 64      # xp row: head slack + one (h,w) plane + tail slack
VPW = 64 + CONVS * 1024 + 64   # vpad row: slack + 6 unpadded slices + slack


# ---------------------------------------------------------------------------
# pass table: 27 offsets -> 14 passes
# ---------------------------------------------------------------------------
def _build_passes():
    passes = []
    for dt in (-1, 0, 1):
        for dh in (-1, 0, 1):
            passes.append(("A", (dt, dh, -1), (dt, dh, 0)))
    # AB pairs chosen so (o2 - o1) is +32 (dh) or +1024 (dt) in flat index
    for dt in (-1, 0, 1):
        passes.append(("AB", (dt, -1, 1), (dt, 0, 1)))
    passes.append(("AB", (-1, 1, 1), (0, 1, 1)))
    passes.append(("SOLO", (1, 1, 1), None))
    return passes

PASSES = _build_passes()      # 14 entries; rounds: [0:4],[4:8],[8:12],[12:14]
ROUNDS = [PASSES[0:4], PASSES[4:8], PASSES[8:12], PASSES[12:14]]


def _host_consts(w_qk, w_v):
    """Prepare weight/constant arrays shipped to every core."""
    import ml_dtypes

    wqk = np.asarray(w_qk, np.float32).reshape(2 * PA * D, K3)
    lhsT_qk = wqk.T.astype(np.float16).copy()              # [27, 128]

    wv = np.asarray(w_v, np.float32).reshape(PA, K3)
    m = wv.max(axis=-1, keepdims=True)
    e = np.exp(wv - m)
    wv_sm = e / e.sum(axis=-1, keepdims=True)
    lhsT_v = wv_sm.T.astype(np.float16).copy()             # [27, 8]

    ones_sim = np.zeros((128, 128), np.float16)
    for j in range(4):
        for rep in range(2):
            for a in range(A):
                col = 32 * j + 8 * rep + a
                for d in range(D):
                    ones_sim[64 * rep + 8 * a + d, col] = 1.0

    ones_solo = np.zeros((128, 32), np.float16)
    for a in range(A):
        for d in range(D):
            ones_solo[8 * a + d, a] = 1.0

    def sg(last):
        g = np.zeros((128, 8), np.float32)
        for j in range(4):
            for rep in range(2):
                if last and (j >= 2 or (j == 1 and rep == 1)):
                    continue
                for a in range(A):
                    g[32 * j + 8 * rep + a, a] = 1.0
        return g

    sg_full = sg(False).astype(ml_dtypes.bfloat16)
    sg_last = sg(True).astype(ml_dtypes.bfloat16)
    return dict(lhsT_qk=lhsT_qk, lhsT_v=lhsT_v, ones_sim=ones_sim,
                ones_solo=ones_solo, sg_full=sg_full, sg_last=sg_last)


# ---------------------------------------------------------------------------
# bass program
# ---------------------------------------------------------------------------
def _win(ap, dims, off):
    """Partition-sliced AP -> same partitions, custom free dims + offset."""
    import concourse.bass as bass
    return bass.AP(tensor=ap.tensor, offset=ap.offset + off,
                   ap=[list(ap.ap[0])] + [list(d) for d in dims])


def build_program():
    import concourse.bass as bass
    import concourse.bacc as bacc
    import concourse.tile as tile
    from concourse import mybir
    from concourse import bass_isa
    from contextlib import ExitStack

    f32, f16, bf16 = mybir.dt.float32, mybir.dt.float16, mybir.dt.bfloat16

    nc = bacc.Bacc("TRN2", target_bir_lowering=False, debug=False,
                   num_devices=8)

    values = nc.declare_dram_parameter("values", [THW], f32, isOutput=False)
    rewards = nc.declare_dram_parameter("rewards", [THW], f32, isOutput=False)
    d_lqk = nc.declare_dram_parameter("lhsT_qk", [K3, 128], f16, isOutput=False)
    d_lv = nc.declare_dram_parameter("lhsT_v", [K3, 8], f16, isOutput=False)
    d_osim = nc.declare_dram_parameter("ones_sim", [128, 128], f16, isOutput=False)
    d_osolo = nc.declare_dram_parameter("ones_solo", [128, 32], f16, isOutput=False)
    d_sgf = nc.declare_dram_parameter("sg_full", [128, 8], bf16, isOutput=False)
    d_sgl = nc.declare_dram_parameter("sg_last", [128, 8], bf16, isOutput=False)
    out = nc.declare_dram_parameter("out", [THW], bf16, isOutput=True)

    Exp = mybir.ActivationFunctionType.Exp

    with tile.TileContext(nc) as tc, ExitStack() as ctx:
        pw = ctx.enter_context(tc.tile_pool(name="w", bufs=1))
        pstage = ctx.enter_context(tc.tile_pool(name="stage", bufs=2))
        pxpad = ctx.enter_context(tc.tile_pool(name="xpad", bufs=1))
        pcols = ctx.enter_context(tc.tile_pool(name="cols", bufs=1))
        pq2 = ctx.enter_context(tc.tile_pool(name="q2", bufs=2))
        pk2 = ctx.enter_context(tc.tile_pool(name="k2", bufs=2))
        pvpad = ctx.enter_context(tc.tile_pool(name="vpad", bufs=1))
        pvn = ctx.enter_context(tc.tile_pool(name="vn", bufs=1))
        pprod = ctx.enter_context(tc.tile_pool(name="prod", bufs=2))
        pE = ctx.enter_context(tc.tile_pool(name="E", bufs=2))
        pG = ctx.enter_context(tc.tile_pool(name="G", bufs=2))
        psmall = ctx.enter_context(tc.tile_pool(name="small", bufs=1))
        pout = ctx.enter_context(tc.tile_pool(name="outp", bufs=1))

        ppqk = ctx.enter_context(tc.tile_pool(name="ppqk", bufs=2, space="PSUM"))
        ppv = ctx.enter_context(tc.tile_pool(name="ppv", bufs=1, space="PSUM"))
        ppsim = ctx.enter_context(tc.tile_pool(name="ppsim", bufs=2, space="PSUM"))
        ppS = ctx.enter_context(tc.tile_pool(name="ppS", bufs=1, space="PSUM"))
        ppqv = ctx.enter_context(tc.tile_pool(name="ppqv", bufs=1, space="PSUM"))

        # --- constants into SBUF ---
        t_lqk = pw.tile([K3, 128], f16)
        nc.sync.dma_start(t_lqk[:], d_lqk[:])
        t_lv = pw.tile([K3, 8], f16)
        nc.sync.dma_start(t_lv[:], d_lv[:])
        t_osim = pw.tile([128, 128], f16)
        nc.sync.dma_start(t_osim[:], d_osim[:])
        t_osolo = pw.tile([128, 32], f16)
        nc.sync.dma_start(t_osolo[:], d_osolo[:])
        t_sgf = pw.tile([128, 8], bf16)
        nc.sync.dma_start(t_sgf[:], d_sgf[:])
        t_sgl = pw.tile([128, 8], bf16)
        nc.sync.dma_start(t_sgl[:], d_sgl[:])

        # persistent tiles (zeroed once; per-slab writes cover the interiors,
        # borders/slack/dead rows stay zero)
        vn = [pvn.tile([128, SLABP], bf16, name=f"vn{r}", tag=f"vn{r}")
              for r in range(4)]
        for v_ in vn:
            nc.gpsimd.memset(v_[:], 0.0)
        k2s = [pk2.tile([128, PADVOL], f16, name=f"k2_{i}", tag=f"k2_{i}")
               for i in range(2)]
        for k_ in k2s:
            nc.vector.memset(k_[:], 0.0)
        vpA = pvpad.tile([16, VPW], bf16, name="vpA", tag="vpA")
        vpB = pvpad.tile([16, VPW], bf16, name="vpB", tag="vpB")
        nc.vector.memset(vpA[:], 0.0)
        nc.vector.memset(vpB[:], 0.0)

        for s in range(NSLAB):
            xt0 = 4 * s - 2
            vt0, vt1 = max(0, xt0), min(T, xt0 + 8)
            nt, tl0 = vt1 - vt0, vt0 - xt0

            # --- staging [t-slices on partitions] -> xp (unpadded + slack) ---
            stv = pstage.tile([8, 1024], f32, tag="stv")
            str_ = pstage.tile([8, 1024], f32, tag="str")
            for dram, st in ((values, stv), (rewards, str_)):
                srcd = bass.AP(tensor=dram, offset=vt0 * 1024,
                               ap=[[1024, nt], [1, 1024]])
                nc.sync.dma_start(st[0:nt, :], srcd)

            xsum = pstage.tile([8, 1024], f16, tag="xsum")
            nc.vector.tensor_add(xsum[0:nt, :], stv[0:nt, :], str_[0:nt, :])

            xp = pxpad.tile([8, PADHW], f16)
            nc.gpsimd.memset(xp[:], 0.0)
            nc.sync.dma_start(
                _win(xp[tl0:tl0 + nt, :], [[34, 32], [1, 32]], 35),
                xsum[0:nt, :])

            # --- cols: 27 strided-window DMAs (borders come from xp pad) ---
            cols = pcols.tile([K3, COLSN], f16)
            for j in range(K3):
                dt, dh, dw = j // 9 - 1, (j // 3) % 3 - 1, j % 3 - 1
                srcx = _win(xp[1 + dt:7 + dt, :], [[34, 32], [1, 32]],
                            (1 + dh) * 34 + (1 + dw))
                nc.sync.dma_start(_win(cols[j:j + 1, :], [[1, COLSN]], 0),
                                  srcx)

            # --- conv + evictions ---
            q2 = pq2.tile([128, SLABP], f16)
            k2 = k2s[s % 2]
            if s == NSLAB - 1:
                # slice u=5 (t=32) skipped by eviction; clear stale data
                nc.vector.memset(k2[:, 5 * PADHW:6 * PADHW], 0.0)
                nc.vector.memset(vpA[0:8, 64 + 5 * 1024:64 + 6 * 1024], 0.0)

            for cc in range(2 * CONVS):
                u, hh = cc // 2, cc % 2
                tc_glob = 4 * s - 1 + u
                rhs = cols[:, cc * 512:(cc + 1) * 512]
                qkp = ppqk.tile([128, 512], f32)
                nc.tensor.matmul(qkp[:], t_lqk[:], rhs, start=True, stop=True)
                vp = ppv.tile([8, 512], f32)
                nc.tensor.matmul(vp[:], t_lv[:], rhs, start=True, stop=True)

                if 0 <= tc_glob < T:
                    pad_off = u * PADHW + (1 + 16 * hh) * 34 + 1
                    srck = _win(qkp[64:128, :], [[32, 16], [1, 32]], 0)
                    nc.scalar.copy(_win(k2[0:64, :], [[34, 16], [1, 32]], pad_off), srck)
                    nc.scalar.copy(_win(k2[64:128, :], [[34, 16], [1, 32]], pad_off - 1), srck)
                    nc.scalar.copy(vpA[0:8, 64 + cc * 512:64 + (cc + 1) * 512],
                                   vp[:])
                if 1 <= u < 5:
                    qoff = (u - 1) * 1024 + hh * 512
                    srcq = qkp[0:64, :]
                    nc.scalar.copy(q2[0:64, qoff:qoff + 512], srcq)
                    nc.scalar.copy(q2[64:128, qoff:qoff + 512], srcq)

            # --- v replicas (rows 8:16 shifted +1 / +32) ---
            nc.sync.dma_start(_win(vpA[8:16, :], [[1, VPW - 1]], 0),
                              _win(vpA[0:8, :], [[1, VPW - 1]], 1))
            nc.sync.dma_start(_win(vpB[0:8, :], [[1, VPW]], 0),
                              _win(vpA[0:8, :], [[1, VPW]], 0))
            nc.sync.dma_start(_win(vpB[8:16, :], [[1, VPW - 32]], 0),
                              _win(vpA[0:8, :], [[1, VPW - 32]], 32))

            # --- vn gathers: contiguous shifted reads + border fixups ---
            def vdelta(o):
                dt_, dh_, dw_ = o
                return (1 + dt_) * 1024 + dh_ * 32 + dw_

            for p_idx, (kind, o1, o2) in enumerate(PASSES):
                r, jj = p_idx // 4, p_idx % 4
                row = 32 * jj
                if kind == "A":
                    nc.sync.dma_start(
                        _win(vn[r][row:row + 16, :], [[1, SLABP]], 0),
                        _win(vpA[0:16, :], [[1, SLABP]], 64 + vdelta(o1)))
                    groups = [(0, o1), (8, o2)]
                elif kind == "AB" and o2[0] == o1[0]:   # dh-pair, delta +32
                    nc.sync.dma_start(
                        _win(vn[r][row:row + 16, :], [[1, SLABP]], 0),
                        _win(vpB[0:16, :], [[1, SLABP]], 64 + vdelta(o1)))
                    groups = [(0, o1), (8, o2)]
                elif kind == "AB":                      # dt-pair: two reads
                    nc.sync.dma_start(
                        _win(vn[r][row:row + 8, :], [[1, SLABP]], 0),
                        _win(vpA[0:8, :], [[1, SLABP]], 64 + vdelta(o1)))
                    nc.sync.dma_start(
                        _win(vn[r][row + 8:row + 16, :], [[1, SLABP]], 0),
                        _win(vpA[0:8, :], [[1, SLABP]], 64 + vdelta(o2)))
                    groups = [(0, o1), (8, o2)]
                else:                                   # SOLO
                    nc.sync.dma_start(
                        _win(vn[r][row:row + 8, :], [[1, SLABP]], 0),
                        _win(vpA[0:8, :], [[1, SLABP]], 64 + vdelta(o1)))
                    groups = [(0, o1)]
                # border fixups: emit per border type over 32-aligned groups
                for sign, dims in ((1, [[1024, TS], [1, 32]]),
                                   (0, [[1024, TS], [32, 32]])):
                    for edge in (-1, 1):
                        need = [ro for ro, o in groups if o[1 if sign else 2] == edge]
                        if not need:
                            continue
                        assert need[0] == 0, "rep1-only border not 32-aligned"
                        nrows = 16 if len(need) == 2 else 8
                        off = ((31 if edge == 1 else 0) * 32 if sign
                               else (31 if edge == 1 else 0))
                        nc.gpsimd.memset(
                            _win(vn[r][row:row + nrows, :], dims, off), 0.0)

            # --- sim + softmax + qv ---
            tS = psmall.tile([8, SLABP], bf16, tag="tS")
            tQ = psmall.tile([8, SLABP], bf16, tag="tQ")
            tR = psmall.tile([8, SLABP], bf16, tag="tR")
            for c in range(NCHUNK):
                tq, hh = c // 2, c % 2
                simp = ppsim.tile([128, 512], f32)
                Spp = ppS.tile([8, 512], f32)
                qvpp = ppqv.tile([8, 512], f32)
                for r, rnd in enumerate(ROUNDS):
                    for jj, (kind, o1, o2) in enumerate(rnd):
                        prod = pprod.tile([128, 512], f16)
                        q2c = q2[:, c * 512:(c + 1) * 512]

                        def kwin(ap, o):
                            dt, dh, dw = o
                            off = ((tq + 1 + dt) * PADHW
                                   + (1 + 16 * hh + dh) * 34 + 1 + dw)
                            return _win(ap, [[34, 16], [1, 32]], off)

                        if kind == "A":
                            nc.vector.tensor_mul(prod[:], q2c, kwin(k2[:, :], o1))
                        elif kind == "AB":
                            nc.vector.tensor_mul(
                                prod[0:64, :], q2[0:64, c * 512:(c + 1) * 512],
                                kwin(k2[0:64, :], o1))
                            nc.vector.tensor_mul(
                                prod[64:128, :], q2[0:64, c * 512:(c + 1) * 512],
                                kwin(k2[0:64, :], o2))
                        else:  # SOLO
                            nc.vector.tensor_mul(
                                prod[0:64, :], q2[0:64, c * 512:(c + 1) * 512],
                                kwin(k2[0:64, :], o1))

                        if kind == "SOLO":
                            nc.tensor.matmul(simp[32 * jj:32 * jj + 32, :],
                                             t_osolo[0:64, :], prod[0:64, :],
                                             start=True, stop=True,
                                             tile_position=(0, 32 * jj))
                        else:
                            nc.tensor.matmul(simp[32 * jj:32 * jj + 32, :],
                                             t_osim[:, 32 * jj:32 * jj + 32],
                                             prod[:], start=True, stop=True,
                                             tile_position=(0, 32 * jj))

                    # round 3 only writes partitions 0:64 of simp
                    lo = 128 if r < 3 else 64
                    E = pE.tile([128, 512], bf16)
                    nc.scalar.activation(E[0:lo, :], simp[0:lo, :], Exp)
                    G = pG.tile([128, 512], bf16)
                    nc.vector.tensor_mul(G[0:lo, :], E[0:lo, :],
                                         vn[r][0:lo, c * 512:(c + 1) * 512])
                    sg_t = t_sgf if r < 3 else t_sgl
                    nc.tensor.matmul(Spp[:], sg_t[0:lo, :], E[0:lo, :],
                                     start=(r == 0), stop=(r == 3),
                                     tile_position=(0, 0))
                    nc.tensor.matmul(qvpp[:], sg_t[0:lo, :], G[0:lo, :],
                                     start=(r == 0), stop=(r == 3),
                                     tile_position=(0, 0))

                nc.scalar.copy(tS[:, c * 512:(c + 1) * 512], Spp[:])
                nc.scalar.copy(tQ[:, c * 512:(c + 1) * 512], qvpp[:])

            with nc.allow_low_precision(reason="attn denominator in bf16"):
                nc.vector.reciprocal(tR[:], tS[:])
            qvf = pout.tile([8, SLABP], bf16, tag="qvf")
            nc.vector.tensor_mul(qvf[:], tQ[:], tR[:])
            outt = pout.tile([8, SLABP], bf16, tag="outt")
            nc.gpsimd.partition_all_reduce(outt[:], qvf[:], channels=8,
                                           reduce_op=bass_isa.ReduceOp.max)
            dst = bass.AP(tensor=out, offset=s * SLABP,
                          ap=[[SLABP, 1], [1, SLABP]])
            nc.sync.dma_start(dst, _win(outt[0:1, :], [[1, SLABP]], 0))

    nc.compile()
    return nc


_CACHE = {}


def kernel(values, rewards, w_qk, w_v):
    from concourse.bass_utils import run_bass_kernel_spmd

    if "nc" not in _CACHE:
        _CACHE["nc"] = build_program()
    nc = _CACHE["nc"]

    consts = _host_consts(w_qk, w_v)
    values = np.asarray(values, np.float32).reshape(B, THW)
    rewards = np.asarray(rewards, np.float32).reshape(B, THW)

    core_ids = list(range(8))
    in_maps = [dict(values=values[b], rewards=rewards[b], **consts)
               for b in core_ids]
    res = run_bass_kernel_spmd(nc, in_maps, core_ids)
    out = np.stack([np.asarray(res.results[b]["out"]).astype(np.float32)
                    for b in core_ids])
    return out.reshape(B, P, T, H, W)


if __name__ == "__main__":
    rng = np.random.default_rng(0)
    o = kernel(
        values=rng.standard_normal((B, P, T, H, W), dtype=np.float32),
        rewards=rng.standard_normal((B, P, T, H, W), dtype=np.float32),
        w_qk=rng.standard_normal((2 * PA * D, P, K, K, K),
                                 dtype=np.float32) / np.sqrt(P * K ** 3),
        w_v=rng.standard_normal((PA, P, K, K, K),
                                dtype=np.float32) / np.sqrt(P * K ** 3),
    )
    print(o.shape, o.dtype)


# revision 22
# speedup vs baseline: 1.0527x; 1.0527x over previous
"""Trainium2 Bass kernel for nn_AttentionValueIteration.

Data-parallel over batch B=8: one batch element per NeuronCore (8 cores).

Per-core pipeline (T=H=W=32, A=8, D=8, K=3, P=1), processed in 8 slabs of
4 t-slices:
  1. x = values + rewards -> x_pad [32 h-partitions, (8 t, 32 w)] fp16
     (t halo +-2; out-of-range t slices stay zero).
  2. im2col via 27 shifted SBUF->SBUF DMAs -> cols [27, 6*1024] fp16
     (borders provided by a memset; DMAs copy the valid subwindow).
  3. qk conv + v conv as PE matmuls over cols (contract dim 27).
     Evictions build:
       q2  [128 = 2 replicas x 64ch, 4096]          (fp16)
       k2  [128, 6*34*34]  rows 0:64 = padded k, rows 64:128 = k shifted
           by +1 flat element (so one TT covers offset pairs (dw=-1, dw=0))
       v_pad [8, 6*34*34]  padded v                  (bf16)
  4. sim: 14 "passes" cover the 27 neighborhood offsets (9 single-TT pairs
     via the k2 replica trick, 4 two-TT pairs, 1 solo).  Each pass:
     DVE TT prod = q2 * shifted(k2)  [*, 512-chunk], then a PE matmul with
     a constant block-ones lhsT reduces over d (partition groups) into a
     per-round PSUM tile [128, 512] (4 passes -> 4 disjoint 32-row blocks).
  5. Softmax without max-subtraction (|sim| stays well within exp range):
     ACT evicts PSUM with fused Exp -> E [128, 512] bf16.  S = sum_o E and
     qv_raw = sum_o E*vn via PE matmuls with block-ones lhsT accumulating
     over the 4 rounds in PSUM.  vn (neighborhoods of v) built by 32
     slab-wide DMAs from v_pad into persistent, once-zeroed tiles.
  6. qv = qv_raw * reciprocal(S); final max over the 8 action partitions via
     gpsimd partition_all_reduce; DMA row 0 to the output.
"""
import sys

sys.path.insert(0, "/opt/trn_rl_repo")

import numpy as np

B, P, A, D, K = 8, 1, 8, 8, 3
T, H, W = 32, 32, 32
PA = P * A
THW = T * H * W
K3 = 27

TS = 4                    # output t-slices per slab
NSLAB = T // TS           # 8
CONVS = TS + 2            # conv slices per slab (halo +-1)
SLABP = TS * H * W        # 4096 positions per slab
NCHUNK = SLABP // 512     # 8
PADHW = 34 * 34
PADVOL = CONVS * PADHW    # 6936
COLSN = CONVS * H * W     # 6144
XPW = 64 + 1024 + 64      # xp row: head slack + one (h,w) plane + tail slack
VPW = 64 + CONVS * 1024 + 64   # vpad row: slack + 6 unpadded slices + slack


# ---------------------------------------------------------------------------
# pass table: 27 offsets -> 14 passes
# ---------------------------------------------------------------------------
def _build_passes():
    passes = []
    for dt in (-1, 0, 1):
        for dh in (-1, 0, 1):
            passes.append(("A", (dt, dh, -1), (dt, dh, 0)))
    # AB pairs chosen so (o2 - o1) is +32 (dh) or +1024 (dt) in flat index
    for dt in (-1, 0, 1):
        passes.append(("AB", (dt, -1, 1), (dt, 0, 1)))
    passes.append(("AB", (-1, 1, 1), (0, 1, 1)))
    passes.append(("SOLO", (1, 1, 1), None))
    return passes

PASSES = _build_passes()      # 14 entries; rounds: [0:4],[4:8],[8:12],[12:14]
ROUNDS = [PASSES[0:4], PASSES[4:8], PASSES[8:12], PASSES[12:14]]


def _host_consts(w_qk, w_v):
    """Prepare weight/constant arrays shipped to every core."""
    import ml_dtypes

    wqk = np.asarray(w_qk, np.float32).reshape(2 * PA * D, K3)
    lhsT_qk = wqk.T.astype(np.float16).copy()              # [27, 128]

    wv = np.asarray(w_v, np.float32).reshape(PA, K3)
    m = wv.max(axis=-1, keepdims=True)
    e = np.exp(wv - m)
    wv_sm = e / e.sum(axis=-1, keepdims=True)
    lhsT_v = wv_sm.T.astype(np.float16).copy()             # [27, 8]
    qk_dt = [lhsT_qk[9 * i:9 * i + 9].copy() for i in range(3)]
    v_dt = [lhsT_v[9 * i:9 * i + 9].copy() for i in range(3)]

    ones_sim = np.zeros((128, 128), np.float16)
    for j in range(4):
        for rep in range(2):
            for a in range(A):
                col = 32 * j + 8 * rep + a
                for d in range(D):
                    ones_sim[64 * rep + 8 * a + d, col] = 1.0

    ones_solo = np.zeros((128, 32), np.float16)
    for a in range(A):
        for d in range(D):
            ones_solo[8 * a + d, a] = 1.0

    def sg(last):
        g = np.zeros((128, 8), np.float32)
        for j in range(4):
            for rep in range(2):
                if last and (j >= 2 or (j == 1 and rep == 1)):
                    continue
                for a in range(A):
                    g[32 * j + 8 * rep + a, a] = 1.0
        return g

    sg_full = sg(False).astype(ml_dtypes.bfloat16)
    sg_last = sg(True).astype(ml_dtypes.bfloat16)
    return dict(ones_sim=ones_sim, ones_solo=ones_solo,
                sg_full=sg_full, sg_last=sg_last,
                **{f"qk_dt{i}": qk_dt[i] for i in range(3)},
                **{f"v_dt{i}": v_dt[i] for i in range(3)})


# ---------------------------------------------------------------------------
# bass program
# ---------------------------------------------------------------------------
def _win(ap, dims, off):
    """Partition-sliced AP -> same partitions, custom free dims + offset."""
    import concourse.bass as bass
    return bass.AP(tensor=ap.tensor, offset=ap.offset + off,
                   ap=[list(ap.ap[0])] + [list(d) for d in dims])


def build_program():
    import concourse.bass as bass
    import concourse.bacc as bacc
    import concourse.tile as tile
    from concourse import mybir
    from concourse import bass_isa
    from contextlib import ExitStack

    f32, f16, bf16 = mybir.dt.float32, mybir.dt.float16, mybir.dt.bfloat16

    nc = bacc.Bacc("TRN2", target_bir_lowering=False, debug=False,
                   num_devices=8)

    values = nc.declare_dram_parameter("values", [THW], f32, isOutput=False)
    rewards = nc.declare_dram_parameter("rewards", [THW], f32, isOutput=False)
    d_qk = [nc.declare_dram_parameter(f"qk_dt{i}", [9, 128], f16,
                                      isOutput=False) for i in range(3)]
    d_v = [nc.declare_dram_parameter(f"v_dt{i}", [9, 8], f16,
                                     isOutput=False) for i in range(3)]
    d_osim = nc.declare_dram_parameter("ones_sim", [128, 128], f16, isOutput=False)
    d_osolo = nc.declare_dram_parameter("ones_solo", [128, 32], f16, isOutput=False)
    d_sgf = nc.declare_dram_parameter("sg_full", [128, 8], bf16, isOutput=False)
    d_sgl = nc.declare_dram_parameter("sg_last", [128, 8], bf16, isOutput=False)
    out = nc.declare_dram_parameter("out", [THW], bf16, isOutput=True)

    Exp = mybir.ActivationFunctionType.Exp

    with tile.TileContext(nc) as tc, ExitStack() as ctx:
        pw = ctx.enter_context(tc.tile_pool(name="w", bufs=1))
        pstage = ctx.enter_context(tc.tile_pool(name="stage", bufs=1))
        pxpad = ctx.enter_context(tc.tile_pool(name="xpad", bufs=1))
        pcols = ctx.enter_context(tc.tile_pool(name="cols", bufs=1))
        pq2 = ctx.enter_context(tc.tile_pool(name="q2", bufs=2))
        pk2 = ctx.enter_context(tc.tile_pool(name="k2", bufs=2))
        pvpad = ctx.enter_context(tc.tile_pool(name="vpad", bufs=1))
        pvn = ctx.enter_context(tc.tile_pool(name="vn", bufs=1))
        pprod = ctx.enter_context(tc.tile_pool(name="prod", bufs=2))
        pE = ctx.enter_context(tc.tile_pool(name="E", bufs=2))
        pG = ctx.enter_context(tc.tile_pool(name="G", bufs=2))
        psmall = ctx.enter_context(tc.tile_pool(name="small", bufs=1))
        pout = ctx.enter_context(tc.tile_pool(name="outp", bufs=1))

        ppqk = ctx.enter_context(tc.tile_pool(name="ppqk", bufs=2, space="PSUM"))
        ppv = ctx.enter_context(tc.tile_pool(name="ppv", bufs=1, space="PSUM"))
        ppsim = ctx.enter_context(tc.tile_pool(name="ppsim", bufs=2, space="PSUM"))
        ppS = ctx.enter_context(tc.tile_pool(name="ppS", bufs=1, space="PSUM"))
        ppqv = ctx.enter_context(tc.tile_pool(name="ppqv", bufs=1, space="PSUM"))

        # --- constants into SBUF ---
        t_qk, t_v = [], []
        for i in range(3):
            tq_ = pw.tile([9, 128], f16, name=f"tqk{i}", tag=f"tqk{i}")
            nc.sync.dma_start(tq_[:], d_qk[i][:])
            t_qk.append(tq_)
            tv_ = pw.tile([9, 8], f16, name=f"tv{i}", tag=f"tv{i}")
            nc.sync.dma_start(tv_[:], d_v[i][:])
            t_v.append(tv_)
        t_osim = pw.tile([128, 128], f16)
        nc.sync.dma_start(t_osim[:], d_osim[:])
        t_osolo = pw.tile([128, 32], f16)
        nc.sync.dma_start(t_osolo[:], d_osolo[:])
        t_sgf = pw.tile([128, 8], bf16)
        nc.sync.dma_start(t_sgf[:], d_sgf[:])
        t_sgl = pw.tile([128, 8], bf16)
        nc.sync.dma_start(t_sgl[:], d_sgl[:])

        # persistent tiles (zeroed once; per-slab writes cover the interiors,
        # borders/slack/dead rows stay zero)
        vn = [pvn.tile([128, SLABP], bf16, name=f"vn{r}", tag=f"vn{r}")
              for r in range(4)]
        for v_ in vn:
            nc.gpsimd.memset(v_[:], 0.0)
        k2s = [pk2.tile([128, PADVOL], f16, name=f"k2_{i}", tag=f"k2_{i}")
               for i in range(2)]
        for k_ in k2s:
            nc.vector.memset(k_[:], 0.0)
        vpA = pvpad.tile([16, VPW], bf16, name="vpA", tag="vpA")
        vpB = pvpad.tile([16, VPW], bf16, name="vpB", tag="vpB")
        nc.vector.memset(vpA[:], 0.0)
        nc.vector.memset(vpB[:], 0.0)

        for s in range(NSLAB):
            xt0 = 4 * s - 2
            vt0, vt1 = max(0, xt0), min(T, xt0 + 8)
            nt, tl0 = vt1 - vt0, vt0 - xt0

            # --- staging [t-slices on partitions] -> xp (unpadded + slack) ---
            stv = pstage.tile([8, 1024], f32, tag="stv")
            str_ = pstage.tile([8, 1024], f32, tag="str")
            for dram, st in ((values, stv), (rewards, str_)):
                srcd = bass.AP(tensor=dram, offset=vt0 * 1024,
                               ap=[[1024, nt], [1, 1024]])
                nc.sync.dma_start(st[0:nt, :], srcd)

            xsum = pstage.tile([8, 1024], f16, tag="xsum")
            nc.vector.tensor_add(xsum[0:nt, :], stv[0:nt, :], str_[0:nt, :])

            xp = pxpad.tile([8, PADHW], f16)
            nc.gpsimd.memset(xp[:], 0.0)
            nc.sync.dma_start(
                _win(xp[tl0:tl0 + nt, :], [[34, 32], [1, 32]], 35),
                xsum[0:nt, :])

            # --- cols9: 9 padded-flat rows (dh, dw); dt becomes a free
            # offset of +/-1156 in the conv rhs windows ---
            cols = pcols.tile([9, 9350], f16)
            for j9 in range(9):
                dh, dw = j9 // 3 - 1, j9 % 3 - 1
                d9 = dh * 34 + dw
                nc.sync.dma_start(
                    _win(cols[j9:j9 + 1, :], [[1, 8 * PADHW]], 64 - d9),
                    _win(xp[0:8, :], [[1, PADHW]], 0))

            # --- conv + evictions ---
            q2 = pq2.tile([128, SLABP], f16)
            k2 = k2s[s % 2]
            if s == NSLAB - 1:
                # slice u=5 (t=32) skipped by eviction; clear stale data
                nc.vector.memset(k2[:, 5 * PADHW:6 * PADHW], 0.0)
                nc.vector.memset(vpA[0:8, 64 + 5 * 1024:64 + 6 * 1024], 0.0)

            for cc in range(2 * CONVS):
                u, hh = cc // 2, cc % 2
                tc_glob = 4 * s - 1 + u
                qkp = ppqk.tile([128, 512], f32)
                vp = ppv.tile([8, 512], f32)
                for i in range(3):
                    rhs = _win(cols[0:9, :], [[34, 16], [1, 32]],
                               64 + (u + i) * PADHW + (1 + 16 * hh) * 34 + 1)
                    nc.tensor.matmul(qkp[:], t_qk[i][:], rhs,
                                     start=(i == 0), stop=(i == 2),
                                     tile_position=(0, 0))
                    nc.tensor.matmul(vp[:], t_v[i][:], rhs,
                                     start=(i == 0), stop=(i == 2),
                                     tile_position=(0, 0))

                if 0 <= tc_glob < T:
                    pad_off = u * PADHW + (1 + 16 * hh) * 34 + 1
                    srck = _win(qkp[64:128, :], [[32, 16], [1, 32]], 0)
                    nc.scalar.copy(_win(k2[0:64, :], [[34, 16], [1, 32]], pad_off), srck)
                    nc.scalar.copy(_win(k2[64:128, :], [[34, 16], [1, 32]], pad_off - 1), srck)
                    nc.scalar.copy(vpA[0:8, 64 + cc * 512:64 + (cc + 1) * 512],
                                   vp[:])
                if 1 <= u < 5:
                    qoff = (u - 1) * 1024 + hh * 512
                    srcq = qkp[0:64, :]
                    nc.scalar.copy(q2[0:64, qoff:qoff + 512], srcq)
                    nc.scalar.copy(q2[64:128, qoff:qoff + 512], srcq)

            # --- v replicas (rows 8:16 shifted +1 / +32) ---
            nc.sync.dma_start(_win(vpA[8:16, :], [[1, VPW - 1]], 0),
                              _win(vpA[0:8, :], [[1, VPW - 1]], 1))
            nc.sync.dma_start(_win(vpB[0:8, :], [[1, VPW]], 0),
                              _win(vpA[0:8, :], [[1, VPW]], 0))
            nc.sync.dma_start(_win(vpB[8:16, :], [[1, VPW - 32]], 0),
                              _win(vpA[0:8, :], [[1, VPW - 32]], 32))

            # --- vn gathers: contiguous shifted reads + border fixups ---
            def vdelta(o):
                dt_, dh_, dw_ = o
                return (1 + dt_) * 1024 + dh_ * 32 + dw_

            for p_idx, (kind, o1, o2) in enumerate(PASSES):
                r, jj = p_idx // 4, p_idx % 4
                row = 32 * jj
                if kind == "A":
                    nc.sync.dma_start(
                        _win(vn[r][row:row + 16, :], [[1, SLABP]], 0),
                        _win(vpA[0:16, :], [[1, SLABP]], 64 + vdelta(o1)))
                    groups = [(0, o1), (8, o2)]
                elif kind == "AB" and o2[0] == o1[0]:   # dh-pair, delta +32
                    nc.sync.dma_start(
                        _win(vn[r][row:row + 16, :], [[1, SLABP]], 0),
                        _win(vpB[0:16, :], [[1, SLABP]], 64 + vdelta(o1)))
                    groups = [(0, o1), (8, o2)]
                elif kind == "AB":                      # dt-pair: two reads
                    nc.sync.dma_start(
                        _win(vn[r][row:row + 8, :], [[1, SLABP]], 0),
                        _win(vpA[0:8, :], [[1, SLABP]], 64 + vdelta(o1)))
                    nc.sync.dma_start(
                        _win(vn[r][row + 8:row + 16, :], [[1, SLABP]], 0),
                        _win(vpA[0:8, :], [[1, SLABP]], 64 + vdelta(o2)))
                    groups = [(0, o1), (8, o2)]
                else:                                   # SOLO
                    nc.sync.dma_start(
                        _win(vn[r][row:row + 8, :], [[1, SLABP]], 0),
                        _win(vpA[0:8, :], [[1, SLABP]], 64 + vdelta(o1)))
                    groups = [(0, o1)]
                # border fixups: emit per border type over 32-aligned groups
                for sign, dims in ((1, [[1024, TS], [1, 32]]),
                                   (0, [[1024, TS], [32, 32]])):
                    for edge in (-1, 1):
                        need = [ro for ro, o in groups if o[1 if sign else 2] == edge]
                        if not need:
                            continue
                        assert need[0] == 0, "rep1-only border not 32-aligned"
                        nrows = 16 if len(need) == 2 else 8
                        off = ((31 if edge == 1 else 0) * 32 if sign
                               else (31 if edge == 1 else 0))
                        nc.gpsimd.memset(
                            _win(vn[r][row:row + nrows, :], dims, off), 0.0)

            # --- sim + softmax + qv ---
            tS = psmall.tile([8, SLABP], bf16, tag="tS")
            tQ = psmall.tile([8, SLABP], bf16, tag="tQ")
            tR = psmall.tile([8, SLABP], bf16, tag="tR")
            for c in range(NCHUNK):
                tq, hh = c // 2, c % 2
                simp = ppsim.tile([128, 512], f32)
                Spp = ppS.tile([8, 512], f32)
                qvpp = ppqv.tile([8, 512], f32)
                for r, rnd in enumerate(ROUNDS):
                    for jj, (kind, o1, o2) in enumerate(rnd):
                        prod = pprod.tile([128, 512], f16)
                        q2c = q2[:, c * 512:(c + 1) * 512]

                        def kwin(ap, o):
                            dt, dh, dw = o
                            off = ((tq + 1 + dt) * PADHW
                                   + (1 + 16 * hh + dh) * 34 + 1 + dw)
                            return _win(ap, [[34, 16], [1, 32]], off)

                        if kind == "A":
                            nc.vector.tensor_mul(prod[:], q2c, kwin(k2[:, :], o1))
                        elif kind == "AB":
                            nc.vector.tensor_mul(
                                prod[0:64, :], q2[0:64, c * 512:(c + 1) * 512],
                                kwin(k2[0:64, :], o1))
                            nc.vector.tensor_mul(
                                prod[64:128, :], q2[0:64, c * 512:(c + 1) * 512],
                                kwin(k2[0:64, :], o2))
                        else:  # SOLO
                            nc.vector.tensor_mul(
                                prod[0:64, :], q2[0:64, c * 512:(c + 1) * 512],
                                kwin(k2[0:64, :], o1))

                        if kind == "SOLO":
                            nc.tensor.matmul(simp[32 * jj:32 * jj + 32, :],
                                             t_osolo[0:64, :], prod[0:64, :],
                                             start=True, stop=True,
                                             tile_position=(0, 32 * jj))
                        else:
                            nc.tensor.matmul(simp[32 * jj:32 * jj + 32, :],
                                             t_osim[:, 32 * jj:32 * jj + 32],
                                             prod[:], start=True, stop=True,
                                             tile_position=(0, 32 * jj))

                    # round 3 only writes partitions 0:64 of simp
                    lo = 128 if r < 3 else 64
                    E = pE.tile([128, 512], bf16)
                    nc.scalar.activation(E[0:lo, :], simp[0:lo, :], Exp)
                    G = pG.tile([128, 512], bf16)
                    nc.vector.tensor_mul(G[0:lo, :], E[0:lo, :],
                                         vn[r][0:lo, c * 512:(c + 1) * 512])
                    sg_t = t_sgf if r < 3 else t_sgl
                    nc.tensor.matmul(Spp[:], sg_t[0:lo, :], E[0:lo, :],
                                     start=(r == 0), stop=(r == 3),
                                     tile_position=(0, 0))
                    nc.tensor.matmul(qvpp[:], sg_t[0:lo, :], G[0:lo, :],
                                     start=(r == 0), stop=(r == 3),
                                     tile_position=(0, 0))

                nc.scalar.copy(tS[:, c * 512:(c + 1) * 512], Spp[:])
                nc.scalar.copy(tQ[:, c * 512:(c + 1) * 512], qvpp[:])

            with nc.allow_low_precision(reason="attn denominator in bf16"):
                nc.vector.reciprocal(tR[:], tS[:])
            nc.vector.tensor_mul(tQ[:], tQ[:], tR[:])
            outt = pout.tile([8, SLABP], bf16, tag="outt")
            nc.gpsimd.partition_all_reduce(outt[:], tQ[:], channels=8,
                                           reduce_op=bass_isa.ReduceOp.max)
            dst = bass.AP(tensor=out, offset=s * SLABP,
                          ap=[[SLABP, 1], [1, SLABP]])
            nc.sync.dma_start(dst, _win(outt[0:1, :], [[1, SLABP]], 0))

    nc.compile()
    return nc


_CACHE = {}


def kernel(values, rewards, w_qk, w_v):
    from concourse.bass_utils import run_bass_kernel_spmd

    if "nc" not in _CACHE:
        _CACHE["nc"] = build_program()
    nc = _CACHE["nc"]

    consts = _host_consts(w_qk, w_v)
    values = np.asarray(values, np.float32).reshape(B, THW)
    rewards = np.asarray(rewards, np.float32).reshape(B, THW)

    core_ids = list(range(8))
    in_maps = [dict(values=values[b], rewards=rewards[b], **consts)
               for b in core_ids]
    res = run_bass_kernel_spmd(nc, in_maps, core_ids)
    out = np.stack([np.asarray(res.results[b]["out"]).astype(np.float32)
                    for b in core_ids])
    return out.reshape(B, P, T, H, W)


if __name__ == "__main__":
    rng = np.random.default_rng(0)
    o = kernel(
        values=rng.standard_normal((B, P, T, H, W), dtype=np.float32),
        rewards=rng.standard_normal((B, P, T, H, W), dtype=np.float32),
        w_qk=rng.standard_normal((2 * PA * D, P, K, K, K),
                                 dtype=np.float32) / np.sqrt(P * K ** 3),
        w_v=rng.standard_normal((PA, P, K, K, K),
                                dtype=np.float32) / np.sqrt(P * K ** 3),
    )
    print(o.shape, o.dtype)
